# revision 5
# baseline (speedup 1.0000x reference)
"""Decade-weighted-loss kernel for Trainium2 (8 NeuronCores, SPMD).

Math: per batch row b, with d_t = clip(floor(|y_true|), 0, 63):
  counts c[b,k] = #{t : d_t = k},  S[b,k] = sum_{t in bin k} (y_pred-y_true)^2
  result = sqrt( (sum_{b,k: c>0} S/c) / (#nonempty bins) )

Device strategy (data-parallel over batch, 8 rows per core):
  Split d = 4h + l (h in 0..15, l in 0..3).  Per element build fp16 one-hot
  features; the tensor engine accumulates a Gram matrix per 8-column group:
    psum[(hslot,c), (fslot,c')] += sum_p lhsT[p, hslot, c] * rhs[p, fslot, c']
  lhsT slots: U_0..U_15  (U_h = [h==hh])
  rhs  slots: V_0..V_3, VL_1..VL_3, loss  (V_l = [l==ll], VL_l = V_l*loss;
  VL_0 is recovered on the host as loss-column minus the other VLs).
  The c==c' diagonal blocks hold the per-(h,l) counts/loss-sums; the tiny
  final scalar reduction happens on the host (the "all-reduce" over 8 cores).
"""

import sys

sys.path.insert(0, "/opt/trn_rl_repo")

import numpy as np

B, T = 64, 524288
NCORES, P = 8, 128
BL = B // NCORES          # rows per core
COLS = T // P             # free-dim columns per row (4096)
TF = 1024                 # columns per SBUF tile
NH, NV = 16, 4            # d = NV*h + l split
GC = 128 // NH            # columns per matmul group (8)
NRF = 2 * NV              # rhs slots
OUTF = NRF * GC           # psum free size (64)

# engine/config knobs (validated with TimelineSim + hardware)
N_GP_U = 2                # U compares moved to gpsimd
DIFF_ENGINE = "gpsimd"
D16_ACT = True            # d16/h16 conversions on the scalar engine
FLOOR_BIAS = -0.5         # HW f32->int16 convert is round-to-nearest

_CACHE = {}


def build_nc(bl=BL, cols=COLS, tf=TF, floor_bias=FLOOR_BIAS,
             d16_act=D16_ACT, diff_engine=DIFF_ENGINE, n_gp_u=N_GP_U):
    import concourse.tile as tile
    import concourse.mybir as mybir
    from concourse import bacc

    op = mybir.AluOpType
    fn = mybir.ActivationFunctionType
    f32, i16 = mybir.dt.float32, mybir.dt.int16
    f16 = mybir.dt.float16

    nc = bacc.Bacc("TRN2", target_bir_lowering=False)
    yt_d = nc.dram_tensor("y_true", [bl, P, cols], f32, kind="ExternalInput")
    yp_d = nc.dram_tensor("y_pred", [bl, P, cols], f32, kind="ExternalInput")
    out_d = nc.dram_tensor("out", [bl, P, OUTF], f32, kind="ExternalOutput")
    n_tiles = cols // tf
    n_groups = tf // GC

    with tile.TileContext(nc) as tc:
        with (
            tc.tile_pool(name="const", bufs=1) as const_pool,
            tc.tile_pool(name="io", bufs=3) as io_pool,
            tc.tile_pool(name="feat", bufs=2) as feat_pool,
            tc.tile_pool(name="psum", bufs=2, space="PSUM") as psum_pool,
            tc.tile_pool(name="res", bufs=2) as res_pool,
        ):
            bias_t = const_pool.tile([P, 1], f32)
            nc.vector.memset(bias_t[:], floor_bias)
            for r in range(bl):
                ps = psum_pool.tile([P, OUTF], f32)
                for ti in range(n_tiles):
                    j0 = ti * tf
                    ytt = io_pool.tile([P, tf], f32, tag="ytt")
                    nc.sync.dma_start(ytt[:], yt_d[r, :, j0:j0 + tf])
                    ypt = io_pool.tile([P, tf], f32, tag="ypt")
                    nc.sync.dma_start(ypt[:], yp_d[r, :, j0:j0 + tf])

                    a = feat_pool.tile([P, tf], f32, tag="a")
                    nc.scalar.activation(a[:], ytt[:], func=fn.Abs)
                    d16 = feat_pool.tile([P, tf], i16, tag="d16")
                    h16 = feat_pool.tile([P, tf], i16, tag="h16")
                    if d16_act:
                        nc.scalar.activation(d16[:], a[:], func=fn.Identity,
                                             bias=bias_t[:])
                        nc.scalar.activation(h16[:], a[:], func=fn.Identity,
                                             scale=1.0 / NV, bias=bias_t[:])
                    else:
                        nc.vector.tensor_scalar(d16[:], a[:], floor_bias, None,
                                                op0=op.add)
                        nc.vector.tensor_scalar(h16[:], d16[:], 2, None,
                                                op0=op.logical_shift_right)
                    l16 = feat_pool.tile([P, tf], i16, tag="l16")
                    nc.vector.tensor_scalar(l16[:], d16[:], NV - 1, None,
                                            op0=op.bitwise_and)

                    diff = feat_pool.tile([P, tf], f16, tag="diff")
                    getattr(nc, diff_engine).tensor_tensor(
                        diff[:], ypt[:], ytt[:], op=op.subtract)

                    # group-blocked views (matmul operands must be single
                    # free-dim slices, so features are packed per group)
                    h16g = h16[:].rearrange("p (g c) -> p g c", c=GC)
                    l16g = l16[:].rearrange("p (g c) -> p g c", c=GC)
                    diffg = diff[:].rearrange("p (g c) -> p g c", c=GC)

                    U = feat_pool.tile([P, n_groups, NH * GC], f16, tag="U")
                    for hh in range(NH):
                        eng = nc.gpsimd if hh < n_gp_u else nc.vector
                        eng.tensor_scalar(U[:, :, hh * GC:(hh + 1) * GC],
                                          h16g, hh, None, op0=op.is_equal)

                    VV = feat_pool.tile([P, n_groups, NRF * GC], f16, tag="VV")
                    lossg = VV[:, :, (NRF - 1) * GC:NRF * GC]
                    nc.scalar.activation(lossg, diffg, func=fn.Square)
                    for ll in range(NV):
                        nc.vector.tensor_scalar(VV[:, :, ll * GC:(ll + 1) * GC],
                                                l16g, ll, None, op0=op.is_equal)
                    for ll in range(1, NV):
                        nc.vector.tensor_tensor(
                            VV[:, :, (NV - 1 + ll) * GC:(NV + ll) * GC],
                            VV[:, :, ll * GC:(ll + 1) * GC],
                            lossg, op=op.mult)

                    for g in range(n_groups):
                        first = ti == 0 and g == 0
                        last = ti == n_tiles - 1 and g == n_groups - 1
                        nc.tensor.matmul(ps[:], U[:, g, :], VV[:, g, :],
                                         start=first, stop=last)

                res = res_pool.tile([P, OUTF], f32, tag="res")
                nc.vector.tensor_copy(res[:], ps[:])
                nc.sync.dma_start(out_d[r, :, :], res[:])

    nc.finalize()
    return nc


def host_reduce(outs):
    """Recover per-row C,S with drop-one algebra; return the final scalar.

    psum layout: m = h*GC + c, n = fslot*GC + c.
    fslot 0..3 = V counts, 4..6 = VL for l = 1..3, 7 = loss (all l).
    """
    num = 0.0
    den = 0.0
    for o in outs:
        for r in range(o.shape[0]):
            ps = o[r].astype(np.float64)
            G = np.zeros((NH, NRF))
            for c in range(GC):
                G += ps[c::GC, c::GC]
            Cp = G[:, 0:NV].copy()
            Sp = np.zeros((NH, NV))
            Sp[:, 1:] = G[:, NV:NV + NV - 1]
            Sp[:, 0] = G[:, NRF - 1] - Sp[:, 1:].sum(axis=1)
            mask = Cp > 0
            num += (Sp[mask] / Cp[mask]).sum()
            den += mask.sum()
    return np.float32(np.sqrt(num / den))


def make_in_maps(y_pred, y_true):
    yp = np.ascontiguousarray(y_pred, dtype=np.float32).reshape(B, T)
    yt = np.ascontiguousarray(y_true, dtype=np.float32).reshape(B, T)
    in_maps = []
    for c in range(NCORES):
        sl = slice(c * BL, (c + 1) * BL)
        in_maps.append({
            "y_pred": yp[sl].reshape(BL, P, COLS),
            "y_true": yt[sl].reshape(BL, P, COLS),
        })
    return in_maps


def kernel(y_pred, y_true):
    from concourse.bass_utils import run_bass_kernel_spmd

    if "nc" not in _CACHE:
        _CACHE["nc"] = build_nc()
    nc = _CACHE["nc"]
    in_maps = make_in_maps(y_pred, y_true)
    res = run_bass_kernel_spmd(nc, in_maps, core_ids=list(range(NCORES)))
    return host_reduce([r["out"] for r in res.results])


# revision 9
# speedup vs baseline: 40.4688x; 40.4688x over previous
"""Decade-weighted-loss kernel for Trainium2 (8 NeuronCores, SPMD).

Math: per batch row b, with d_t = clip(floor(|y_true|), 0, 63):
  counts c[b,k] = #{t : d_t = k},  S[b,k] = sum_{t in bin k} (y_pred-y_true)^2
  result = sqrt( (sum_{b,k: c>0} S/c) / (#nonempty bins) )

Device strategy (data-parallel over batch, 8 rows per core):
  Split d = 4h + l (h in 0..15, l in 0..3).  Per element build fp16 one-hot
  features; the tensor engine accumulates a Gram matrix per 8-column group:
    psum[(hslot,c), (fslot,c')] += sum_p lhsT[p, hslot, c] * rhs[p, fslot, c']
  lhsT slots: U_0..U_15  (U_h = [h==hh])
  rhs  slots: V_0..V_3, VL_1..VL_3, loss  (V_l = [l==ll], VL_l = V_l*loss;
  VL_0 is recovered on the host as loss-column minus the other VLs).
  The c==c' diagonal blocks hold the per-(h,l) counts/loss-sums; the tiny
  final scalar reduction happens on the host (the "all-reduce" over 8 cores).
"""

import sys

sys.path.insert(0, "/opt/trn_rl_repo")

import numpy as np

B, T = 64, 524288
NCORES, P = 8, 128
BL = B // NCORES          # rows per core
COLS = T // P             # free-dim columns per row (4096)
TF = 1024                 # columns per SBUF tile
NH, NV = 16, 4            # d = NV*h + l split
GC = 128 // NH            # columns per matmul group (8)
NRF = 2 * NV              # rhs slots
OUTF = NRF * GC           # psum free size (64)

# engine/config knobs (validated with TimelineSim + hardware)
N_GP_U = 2                # U compares moved to gpsimd
DIFF_ENGINE = "gpsimd"
D16_ACT = True            # d16/h16 conversions on the scalar engine
FLOOR_BIAS = -0.5         # HW f32->int16 convert is round-to-nearest

_CACHE = {}


def build_nc(bl=BL, cols=COLS, tf=TF, floor_bias=FLOOR_BIAS,
             d16_act=D16_ACT, diff_engine=DIFF_ENGINE, n_gp_u=N_GP_U,
             reps=1):
    # reps > 1 re-runs the whole pass inside one launch (for timing via
    # wall-clock slope); counts and sums scale by reps, which cancels in
    # S/C, so the result is unchanged.
    import concourse.tile as tile
    import concourse.mybir as mybir
    from concourse import bacc

    op = mybir.AluOpType
    fn = mybir.ActivationFunctionType
    f32, i16 = mybir.dt.float32, mybir.dt.int16
    f16 = mybir.dt.float16

    nc = bacc.Bacc("TRN2", target_bir_lowering=False)
    yt_d = nc.dram_tensor("y_true", [bl, P, cols], f32, kind="ExternalInput")
    yp_d = nc.dram_tensor("y_pred", [bl, P, cols], f32, kind="ExternalInput")
    out_d = nc.dram_tensor("out", [bl, P, OUTF], f32, kind="ExternalOutput")
    n_tiles = cols // tf
    n_groups = tf // GC

    with tile.TileContext(nc) as tc:
        with (
            tc.tile_pool(name="const", bufs=1) as const_pool,
            tc.tile_pool(name="io", bufs=3) as io_pool,
            tc.tile_pool(name="feat", bufs=2) as feat_pool,
            tc.tile_pool(name="psum", bufs=1, space="PSUM") as psum_pool,
            tc.tile_pool(name="res", bufs=2) as res_pool,
        ):
            bias_t = const_pool.tile([P, 1], f32)
            nc.vector.memset(bias_t[:], floor_bias)
            ps_tiles = [psum_pool.tile([P, OUTF], f32, name=f"ps{_r}",
                                       tag=f"ps{_r}") for _r in range(bl)]
            for rep in range(reps):
              for r in range(bl):
                ps = ps_tiles[r]
                for ti in range(n_tiles):
                    j0 = ti * tf
                    ytt = io_pool.tile([P, tf], f32, tag="ytt")
                    nc.sync.dma_start(ytt[:], yt_d[r, :, j0:j0 + tf])
                    ypt = io_pool.tile([P, tf], f32, tag="ypt")
                    nc.sync.dma_start(ypt[:], yp_d[r, :, j0:j0 + tf])

                    a = feat_pool.tile([P, tf], f32, tag="a")
                    nc.scalar.activation(a[:], ytt[:], func=fn.Abs)
                    d16 = feat_pool.tile([P, tf], i16, tag="d16")
                    h16 = feat_pool.tile([P, tf], i16, tag="h16")
                    if d16_act:
                        nc.scalar.activation(d16[:], a[:], func=fn.Identity,
                                             bias=bias_t[:])
                        nc.scalar.activation(h16[:], a[:], func=fn.Identity,
                                             scale=1.0 / NV, bias=bias_t[:])
                    else:
                        nc.vector.tensor_scalar(d16[:], a[:], floor_bias, None,
                                                op0=op.add)
                        nc.vector.tensor_scalar(h16[:], d16[:], 2, None,
                                                op0=op.logical_shift_right)
                    l16 = feat_pool.tile([P, tf], i16, tag="l16")
                    nc.vector.tensor_scalar(l16[:], d16[:], NV - 1, None,
                                            op0=op.bitwise_and)

                    diff = feat_pool.tile([P, tf], f16, tag="diff")
                    getattr(nc, diff_engine).tensor_tensor(
                        diff[:], ypt[:], ytt[:], op=op.subtract)

                    # group-blocked views (matmul operands must be single
                    # free-dim slices, so features are packed per group)
                    h16g = h16[:].rearrange("p (g c) -> p g c", c=GC)
                    l16g = l16[:].rearrange("p (g c) -> p g c", c=GC)
                    diffg = diff[:].rearrange("p (g c) -> p g c", c=GC)

                    U = feat_pool.tile([P, n_groups, NH * GC], f16, tag="U")
                    for hh in range(NH):
                        eng = nc.gpsimd if hh < n_gp_u else nc.vector
                        eng.tensor_scalar(U[:, :, hh * GC:(hh + 1) * GC],
                                          h16g, hh, None, op0=op.is_equal)

                    VV = feat_pool.tile([P, n_groups, NRF * GC], f16, tag="VV")
                    lossg = VV[:, :, (NRF - 1) * GC:NRF * GC]
                    nc.scalar.activation(lossg, diffg, func=fn.Square)
                    for ll in range(NV):
                        nc.vector.tensor_scalar(VV[:, :, ll * GC:(ll + 1) * GC],
                                                l16g, ll, None, op0=op.is_equal)
                    for ll in range(1, NV):
                        nc.vector.tensor_tensor(
                            VV[:, :, (NV - 1 + ll) * GC:(NV + ll) * GC],
                            VV[:, :, ll * GC:(ll + 1) * GC],
                            lossg, op=op.mult)

                    for g in range(n_groups):
                        first = rep == 0 and ti == 0 and g == 0
                        last = (rep == reps - 1 and ti == n_tiles - 1
                                and g == n_groups - 1)
                        nc.tensor.matmul(ps[:], U[:, g, :], VV[:, g, :],
                                         start=first, stop=last)

            for r in range(bl):
                res = res_pool.tile([P, OUTF], f32, tag="res")
                nc.vector.tensor_copy(res[:], ps_tiles[r][:])
                nc.sync.dma_start(out_d[r, :, :], res[:])

    nc.finalize()
    return nc


def host_reduce(outs):
    """Recover per-row C,S with drop-one algebra; return the final scalar.

    psum layout: m = h*GC + c, n = fslot*GC + c.
    fslot 0..3 = V counts, 4..6 = VL for l = 1..3, 7 = loss (all l).
    """
    num = 0.0
    den = 0.0
    for o in outs:
        for r in range(o.shape[0]):
            ps = o[r].astype(np.float64)
            G = np.zeros((NH, NRF))
            for c in range(GC):
                G += ps[c::GC, c::GC]
            Cp = G[:, 0:NV].copy()
            Sp = np.zeros((NH, NV))
            Sp[:, 1:] = G[:, NV:NV + NV - 1]
            Sp[:, 0] = G[:, NRF - 1] - Sp[:, 1:].sum(axis=1)
            mask = Cp > 0
            num += (Sp[mask] / Cp[mask]).sum()
            den += mask.sum()
    return np.float32(np.sqrt(num / den))


def make_in_maps(y_pred, y_true):
    yp = np.ascontiguousarray(y_pred, dtype=np.float32).reshape(B, T)
    yt = np.ascontiguousarray(y_true, dtype=np.float32).reshape(B, T)
    in_maps = []
    for c in range(NCORES):
        sl = slice(c * BL, (c + 1) * BL)
        in_maps.append({
            "y_pred": yp[sl].reshape(BL, P, COLS),
            "y_true": yt[sl].reshape(BL, P, COLS),
        })
    return in_maps


def kernel(y_pred, y_true):
    from concourse.bass_utils import run_bass_kernel_spmd

    if "nc" not in _CACHE:
        _CACHE["nc"] = build_nc()
    nc = _CACHE["nc"]
    in_maps = make_in_maps(y_pred, y_true)
    res = run_bass_kernel_spmd(nc, in_maps, core_ids=list(range(NCORES)))
    return host_reduce([r["out"] for r in res.results])
